# revision 1
# baseline (speedup 1.0000x reference)
"""Trainium2 Bass kernel for nn_Block_19121194402322 (dense_transformer).

Math notes (validated numerically against the reference):
  - The reference einsum 'bnqk,bnvd->bnqd' contracts over BOTH k and v, so
    out[b,n,q,d] = (sum_k softmax(...)[q,k]) * (sum_v v[b,n,v,d]).  Softmax rows
    sum to exactly 1, so the whole Q/K/softmax pipeline is dead code; the
    attention output is the per-head sum of v broadcast over q.
  - After the (non-standard) reshape, head n owns flat sub-rows
    r in [1024n, 1024(n+1)) of (x@Wv).reshape(12288, 64), r = 12 s + c.
    So  w[n*64+d] = sum_{(s,c): (12s+c)//1024 == n} (x@Wv)[s, c*64+d].
    With a 0/1 selector A (rows indexed by (c,n)):  Y = A @ x,  R = Y @ Wv,
    and w is a small gather-sum of 64-wide diagonal blocks of R.
  - LN(out_attn) is therefore one 768-vector per batch element, broadcast
    over the sequence:  a = x + LN1(w).
  - MLP: h = gelu(a@W1 + b1); m = gelu(h@W2 + b2); out = x + LN2(m).

Distribution: pure data-parallel over batch B=8 across the 8 NeuronCores
(one batch element per core); weights replicated.  No collectives.
"""

import numpy as np

S = 1024
E = 768
HID = 1536
HEADS = 12
HD = 64
EPS = 1e-5
P = 128
N_CORES = 8
ACOLS = 256  # selector columns, padded: col = c*16 + n  (c<12, n<12)

_CACHE = {}


def _build_selector_T():
    """A^T with shape (S, ACOLS) fp32; column c*16+n selects tokens s whose
    chunk c belongs to head n, i.e. (12 s + c) // 1024 == n."""
    at = np.zeros((S, ACOLS), np.float32)
    s = np.arange(S)
    for c in range(HEADS):
        n = (HEADS * s + c) // S
        at[s, c * 16 + n] = 1.0
    return at


def _split_multi_waits(m):
    """Hoist all-but-one sync waits of each instruction onto preceding
    single-wait EventSemaphore instructions on the same engine.  Several TPB
    instruction structs (LDWEIGHTS for 4-byte matmuls, ctrl no-operand) carry
    only one sync-wait slot, and walrus codegen errors on more."""
    counter = [0]

    def fix_block(blk):
        out = []
        for inst in blk.get("instructions", []):
            si = inst.get("sync_info")
            waits = (si or {}).get("on_wait") or []
            if si and len(waits) > 1 and inst.get("opcode") != "EventSemaphore":
                for w in waits[:-1]:
                    counter[0] += 1
                    out.append({
                        "debug": inst.get("debug", 0), "engine": inst["engine"],
                        "ins": [], "outs": [], "name": f"I-wsplit-{counter[0]}",
                        "opcode": "EventSemaphore",
                        "sync_info": {"on_update": [], "on_wait": [w]},
                    })
                si["on_wait"] = waits[-1:]
            out.append(inst)
        blk["instructions"] = out
        for sub in blk.get("blocks", []):
            fix_block(sub)

    for fn in m["functions"]:
        for blk in fn["blocks"]:
            fix_block(blk)
    return m


def _build_bass(reps=1):
    import json
    import concourse.bass as bass
    import concourse.mybir as mybir
    import concourse.tile as tile

    f32 = mybir.dt.float32
    f32r = mybir.dt.float32r
    AX = mybir.AxisListType.X
    OP = mybir.AluOpType
    AF = mybir.ActivationFunctionType

    nc = bass.Bass(trn_type="TRN2")

    x_d = nc.declare_dram_parameter("x", [S, E], f32r, isOutput=False)
    wv_d = nc.declare_dram_parameter("Wv", [E, E], f32r, isOutput=False)
    w1_d = nc.declare_dram_parameter("W1", [E, HID], f32r, isOutput=False)
    w2_d = nc.declare_dram_parameter("W2", [HID, E], f32r, isOutput=False)
    at_d = nc.declare_dram_parameter("AT", [S, ACOLS], f32r, isOutput=False)
    idn_d = nc.declare_dram_parameter("IDN", [P, P], f32r, isOutput=False)
    b1_d = nc.declare_dram_parameter("b1", [HID], f32, isOutput=False)
    b2_d = nc.declare_dram_parameter("b2", [E], f32, isOutput=False)
    g1_d = nc.declare_dram_parameter("g1", [E], f32, isOutput=False)
    be1_d = nc.declare_dram_parameter("beta1", [E], f32, isOutput=False)
    g2_d = nc.declare_dram_parameter("g2", [E], f32, isOutput=False)
    be2_d = nc.declare_dram_parameter("beta2", [E], f32, isOutput=False)
    out_d = nc.declare_dram_parameter("out", [S, E], f32, isOutput=True)

    x_v = x_d[:].rearrange("(o p) f -> p o f", p=P)  # (128, 8, 768)
    out_v = out_d[:].rearrange("(o p) f -> p o f", p=P)
    at_v = at_d[:].rearrange("(o p) f -> p o f", p=P)  # (128, 8, 256)
    wv_v = wv_d[:].rearrange("(k p) f -> p k f", p=P)  # (128, 6, 768)
    w1_v = w1_d[:].rearrange("(k p) f -> p k f", p=P)  # (128, 6, 1536)
    w2_v = w2_d[:].rearrange("(k p) f -> p k f", p=P)  # (128, 12, 768)

    KE = E // P      # 6
    KH = HID // P    # 12
    OT = S // P      # 8 token tiles

    with tile.TileContext(nc) as tc:
        with (
            tc.tile_pool(name="w1p", bufs=1) as w1p,
            tc.tile_pool(name="w2p", bufs=1) as w2p,
            tc.tile_pool(name="xg", bufs=1) as xg,        # x then G
            tc.tile_pool(name="wvxt", bufs=1) as wvxt,    # Wv then xT
            tc.tile_pool(name="ytm", bufs=1) as ytm,      # YT then msb
            tc.tile_pool(name="atr", bufs=1) as atr,      # AT then R
            tc.tile_pool(name="bcast", bufs=1) as bcastp,
            tc.tile_pool(name="consts", bufs=1) as consts,
            tc.tile_pool(name="small", bufs=1) as small,
            tc.tile_pool(name="stat", bufs=4) as statp,
            tc.tile_pool(name="xr", bufs=2) as xrpool,
            tc.tile_pool(name="ps", bufs=4, space="PSUM") as psp,
            tc.tile_pool(name="pst", bufs=4, space="PSUM") as pstp,
        ):
            for _rep in range(reps):
                # ---- constant / weight loads -------------------------------
                # Order matters: the cost of a big transfer delays everything
                # issued after it on the DMA engines, so small/early-needed
                # loads go first and W2 (needed only by mm2) is deferred.
                idn = consts.tile([P, P], f32r)
                nc.sync.dma_start(out=idn, in_=idn_d[:])

                at_sb = atr.tile([P, OT, ACOLS], f32r, tag="atr")
                x_sb = xg.tile([P, OT, E], f32r, tag="xg")
                for o in range(OT):
                    nc.sync.dma_start(out=at_sb[:, o, :], in_=at_v[:, o, :])
                    nc.sync.dma_start(out=x_sb[:, o, :], in_=x_v[:, o, :])

                wv_sb = wvxt.tile([P, KE, E], f32r, tag="wvxt")
                for k in range(KE):
                    nc.sync.dma_start(out=wv_sb[:, k, :], in_=wv_v[:, k, :])

                w1_sb = w1p.tile([P, KE, HID], f32r)
                nc.sync.dma_start(out=w1_sb, in_=w1_v)

                w2_sb = w2p.tile([P, KH, E], f32r)
                w2_dma = nc.sync.dma_start(out=w2_sb, in_=w2_v)

                b1col = consts.tile([P, KH], f32)  # b1[j*128+p] at [p, j]
                nc.sync.dma_start(out=b1col, in_=b1_d[:].rearrange("(o p) -> p o", p=P))

                # per-channel vectors in column-chunk layout: v_col[p, j] = v[j*128+p]
                g1col = consts.tile([P, KE], f32)
                be1col = consts.tile([P, KE], f32)
                for j in range(KE):
                    nc.sync.dma_start(out=g1col[:, j:j + 1],
                                      in_=g1_d[j * P:(j + 1) * P].unsqueeze(0))
                    nc.sync.dma_start(out=be1col[:, j:j + 1],
                                      in_=be1_d[j * P:(j + 1) * P].unsqueeze(0))

                b2b = bcastp.tile([P, E], f32)
                nc.gpsimd.dma_start(out=b2b, in_=b2_d[:].partition_broadcast(P))
                g2b = bcastp.tile([P, E], f32)
                nc.gpsimd.dma_start(out=g2b, in_=g2_d[:].partition_broadcast(P))
                be2b = bcastp.tile([P, E], f32)
                nc.gpsimd.dma_start(out=be2b, in_=be2_d[:].partition_broadcast(P))

                eps_sb = consts.tile([P, 1], f32)
                nc.vector.memset(eps_sb, EPS)



                # ---- stage 1: Y^T = x^T @ A^T  (768 x 256) -----------------
                yt_sb = ytm.tile([P, KE, ACOLS], f32r, tag="ytm")
                for i in range(KE):
                    ps = psp.tile([P, 512], f32, tag="ps")
                    for o in range(OT):
                        nc.tensor.matmul(
                            ps[:, :ACOLS],
                            x_sb[:, o, i * P:(i + 1) * P],
                            at_sb[:, o, :],
                            start=(o == 0),
                            stop=(o == OT - 1),
                        )
                    nc.scalar.activation(out=yt_sb[:, i, :], in_=ps[:, :ACOLS], func=AF.Copy)

                # ---- stage 2: w = sum_{c,k} Y^T[:,k,c-group].T @ Wv[:,k,c-block]
                # One PSUM accumulation over 72 small matmuls yields the per-head
                # v-sums w (12, 64) directly -- no gather DMAs needed.
                ps_w = psp.tile([P, 512], f32, tag="ps")
                n_mm = 0
                for k in range(KE):
                    for c in range(HEADS):
                        n_mm += 1
                        nc.tensor.matmul(
                            ps_w[:16, :HD],
                            yt_sb[:, k, c * 16:(c + 1) * 16],
                            wv_sb[:, k, c * HD:(c + 1) * HD],
                            start=(n_mm == 1),
                            stop=(n_mm == KE * HEADS),
                        )
                wacc = small.tile([16, HD], f32)
                nc.scalar.activation(out=wacc, in_=ps_w[:16, :HD], func=AF.Copy)

                # ---- stage 3: LN1 stats + lnvec column ---------------------
                sqw = small.tile([16, HD], f32)
                nc.vector.tensor_mul(sqw, wacc, wacc)
                rsums = small.tile([16, 2], f32)
                nc.vector.tensor_reduce(out=rsums[:, 0:1], in_=wacc, axis=AX, op=OP.add)
                nc.vector.tensor_reduce(out=rsums[:, 1:2], in_=sqw, axis=AX, op=OP.add)

                statrow = small.tile([1, 32], f32)
                nc.gpsimd.dma_start(
                    out=statrow[:, :].rearrange("p (q c) -> p q c", q=16), in_=rsums
                )
                tots = small.tile([1, 2], f32)  # [sum w, sum w^2]
                nc.vector.tensor_reduce(
                    out=tots, in_=statrow[:, :].rearrange("p (q c) -> p c q", q=16),
                    axis=AX, op=OP.add,
                )
                nc.vector.tensor_scalar_mul(tots, tots, 1.0 / E)  # [mu, E[w^2]]
                mu2 = small.tile([1, 1], f32)
                nc.vector.tensor_mul(mu2, tots[:, 0:1], tots[:, 0:1])
                mr = small.tile([32, 2], f32)  # [mu, rstd] written on partition 0
                nc.vector.tensor_sub(mr[:1, 1:2], tots[:, 1:2], mu2)  # var
                nc.scalar.activation(out=mr[:1, 1:2], in_=mr[:1, 1:2], func=AF.Sqrt,
                                     bias=eps_sb[:1])
                nc.vector.reciprocal(mr[:1, 1:2], mr[:1, 1:2])
                nc.vector.tensor_copy(mr[:1, 0:1], tots[:, 0:1])

                # broadcast [mu, rstd] to all 128 partitions via 32-lane shuffles
                mrb = small.tile([P, 2], f32)
                for q in range(4):
                    nc.vector.stream_shuffle(mrb[32 * q:32 * (q + 1), :], mr[:, :],
                                             [0] * 32)

                # lncol[p, j] = w[j*128+p] as a column tile, then normalize+affine
                lncol = small.tile([P, KE], f32)
                from concourse.bass import _add_dep_helper
                last_lncol = None
                for j in range(KE):
                    last_lncol = nc.gpsimd.dma_start(
                        out=lncol[:, j:j + 1],
                        in_=wacc[2 * j:2 * j + 2, :],
                    )
                _add_dep_helper(w2_dma.ins, last_lncol.ins, sync=False,
                                reason="defer W2 load behind the small critical-path DMAs")
                nc.vector.tensor_scalar(lncol, lncol, mrb[:, 0:1], mrb[:, 1:2],
                                        OP.subtract, OP.mult)
                nc.vector.tensor_mul(lncol, lncol, g1col)
                nc.vector.tensor_add(lncol, lncol, be1col)

                # ---- stage 4: aT = x^T + lnvec (PE transpose, DVE add) -----
                xt_sb = wvxt.tile([P, KE, S], f32r, tag="wvxt")
                for j in range(KE):
                    for o in range(OT):
                        pst = pstp.tile([P, P], f32r, tag="pst")
                        nc.tensor.transpose(pst, x_sb[:, o, j * P:(j + 1) * P], idn)
                        nc.scalar.activation(
                            out=xt_sb[:, j, o * P:(o + 1) * P], in_=pst, func=AF.Copy,
                        )
                for j in range(KE):
                    if j % 2 == 0:
                        nc.vector.tensor_scalar_add(
                            xt_sb[:, j, :], xt_sb[:, j, :], lncol[:, j:j + 1]
                        )
                    else:
                        nc.scalar.activation(
                            out=xt_sb[:, j, :], in_=xt_sb[:, j, :],
                            func=AF.Identity, bias=lncol[:, j:j + 1],
                        )

                # ---- stage 5: H^T = W1^T @ aT; G = gelu(H^T + b1) ----------
                g_sb = xg.tile([P, KH, S], f32r, tag="xg")
                for j2 in range(KH):
                    psa = psp.tile([P, 512], f32, tag="ps")
                    psb = psp.tile([P, 512], f32, tag="ps")
                    for k in range(KE):
                        lhs = w1_sb[:, k, j2 * P:(j2 + 1) * P]
                        nc.tensor.matmul(
                            psa, lhs, xt_sb[:, k, 0:512],
                            start=(k == 0), stop=(k == KE - 1),
                        )
                        nc.tensor.matmul(
                            psb, lhs, xt_sb[:, k, 512:1024],
                            start=(k == 0), stop=(k == KE - 1),
                        )
                    nc.scalar.activation(
                        out=g_sb[:, j2, 0:512], in_=psa, func=AF.Gelu,
                        bias=b1col[:, j2:j2 + 1],
                    )
                    nc.scalar.activation(
                        out=g_sb[:, j2, 512:1024], in_=psb, func=AF.Gelu,
                        bias=b1col[:, j2:j2 + 1],
                    )

                # ---- stage 6: m = gelu(G^T @ W2 + b2)  (token-major) -------
                m_sb = ytm.tile([P, OT, E], f32, tag="ytm")
                xrt = {}
                for o in range(OT):
                    xr = xrpool.tile([P, E], f32, tag="xr")
                    nc.sync.dma_start(out=xr, in_=x_v[:, o, :].bitcast(f32))
                    nc.gpsimd.tensor_add(xr, xr, be2b)
                    xrt[o] = xr
                    ps0 = psp.tile([P, 512], f32, tag="ps")
                    ps1 = psp.tile([P, 512], f32, tag="ps")
                    for k in range(KH):
                        lhs = g_sb[:, k, o * P:(o + 1) * P]
                        nc.tensor.matmul(
                            ps0[:, :384], lhs, w2_sb[:, k, 0:384],
                            start=(k == 0), stop=(k == KH - 1),
                        )
                        nc.tensor.matmul(
                            ps1[:, :384], lhs, w2_sb[:, k, 384:768],
                            start=(k == 0), stop=(k == KH - 1),
                        )
                    nc.vector.tensor_add(m_sb[:, o, 0:384], ps0[:, :384], b2b[:, 0:384])
                    nc.vector.tensor_add(m_sb[:, o, 384:768], ps1[:, :384], b2b[:, 384:768])
                    nc.scalar.activation(out=m_sb[:, o, :], in_=m_sb[:, o, :], func=AF.Gelu)

                    # ---- stage 7: LN2 + residual ---------------------------
                    stats = statp.tile([P, 3, 6], f32, tag="st")
                    for sub in range(3):
                        nc.vector.bn_stats(
                            out=stats[:, sub, :], in_=m_sb[:, o, sub * 256:(sub + 1) * 256]
                        )
                    mv = statp.tile([P, 2], f32, tag="mv")
                    nc.vector.bn_aggr(out=mv, in_=stats)
                    rstd = statp.tile([P, 1], f32, tag="rstd")
                    nc.scalar.activation(out=rstd, in_=mv[:, 1:2], func=AF.Sqrt, bias=eps_sb)
                    nc.vector.reciprocal(rstd, rstd)

                    u = m_sb[:, o, :]
                    nc.vector.tensor_scalar(u, u, mv[:, 0:1], rstd, OP.subtract, OP.mult)
                    nc.vector.tensor_mul(u, u, g2b)
                    # split the final add + store by halves so the first half's
                    # write departs while the second half is still computing
                    nc.vector.tensor_add(u[:, 0:384], u[:, 0:384], xrt[o][:, 0:384])
                    nc.sync.dma_start(out=out_v[:, o, 0:384], in_=u[:, 0:384])
                    nc.vector.tensor_add(u[:, 384:768], u[:, 384:768],
                                         xrt[o][:, 384:768])
                    nc.scalar.dma_start(out=out_v[:, o, 384:768], in_=u[:, 384:768])

    m = json.loads(mybir.module_to_json_bytes(nc.m))
    m = _split_multi_waits(m)
    nc.m = mybir.module_from_json_bytes(json.dumps(m).encode())
    return nc


def _get_nc():
    if "nc" not in _CACHE:
        _CACHE["nc"] = _build_bass()
        _CACHE["at"] = _build_selector_T()
    return _CACHE["nc"]


def _run(inputs, trace=False):
    from concourse.bass_utils import run_bass_kernel_spmd

    nc = _get_nc()
    at = _CACHE["at"]

    def f32c(a):
        return np.ascontiguousarray(np.asarray(a), dtype=np.float32)

    shared = {
        "Wv": f32c(inputs["Wv"]),
        "W1": f32c(inputs["W1"]),
        "W2": f32c(inputs["W2"]),
        "AT": at,
        "IDN": np.eye(P, dtype=np.float32),
        "b1": f32c(inputs["b1"]),
        "b2": f32c(inputs["b2"]),
        "g1": f32c(inputs["g1"]),
        "beta1": f32c(inputs["beta1"]),
        "g2": f32c(inputs["g2"]),
        "beta2": f32c(inputs["beta2"]),
    }
    x = f32c(inputs["x"])
    in_maps = [dict(shared, x=x[b]) for b in range(N_CORES)]
    res = run_bass_kernel_spmd(
        nc, in_maps, core_ids=list(range(N_CORES)), trace=trace,
        **({"trace_cores": list(range(N_CORES))} if trace else {}),
    )
    out = np.stack([r["out"] for r in res.results], axis=0)
    return out, res


def kernel(x, Wq=None, Wk=None, Wv=None, W1=None, b1=None, W2=None, b2=None,
           g1=None, beta1=None, g2=None, beta2=None):
    out, _ = _run(dict(x=x, Wv=Wv, W1=W1, b1=b1, W2=W2, b2=b2, g1=g1,
                       beta1=beta1, g2=g2, beta2=beta2))
    return out


def kernel_profiled(**inputs):
    out, res = _run(inputs, trace=True)
    return out, res



# revision 21
# speedup vs baseline: 1.3054x; 1.3054x over previous
"""Trainium2 Bass kernel for nn_Block_19121194402322 (dense_transformer).

Math notes (validated numerically against the reference):
  - The reference einsum 'bnqk,bnvd->bnqd' contracts over BOTH k and v, so
    out[b,n,q,d] = (sum_k softmax(...)[q,k]) * (sum_v v[b,n,v,d]).  Softmax rows
    sum to exactly 1, so the whole Q/K/softmax pipeline is dead code; the
    attention output is the per-head sum of v broadcast over q.
  - After the (non-standard) reshape, head n owns flat sub-rows
    r in [1024n, 1024(n+1)) of (x@Wv).reshape(12288, 64), r = 12 s + c.
    With a 0/1 selector A (rows indexed by (c,n)):  Y = A @ x,  R = Y @ Wv,
    and w (12, 64) accumulates the diagonal blocks of R.
  - LN1(out_attn) is one 768-vector per batch element broadcast over the
    sequence:  a = x + lnvec.  Since lnvec is token-independent,
    h = gelu(a@W1 + b1) = gelu(x@W1 + (W1^T lnvec + b1)) -- the whole LN1
    path folds into the stage-5 gelu BIAS, so the big MLP matmuls depend
    only on x^T and W1 (LN1 off the critical path).
  - MLP: m = gelu(h@W2 + b2); out = x + LN2(m).

All matmul operands are bf16 (PSUM accumulation is fp32); rel err ~3e-3
vs the 2e-2 gate.  Distribution: pure data-parallel over batch B=8 across
8 NeuronCores (one batch element per core); weights replicated, no
collectives.
"""

import numpy as np

S = 1024
E = 768
HID = 1536
HEADS = 12
HD = 64
EPS = 1e-5
P = 128
N_CORES = 8
ACOLS = 192          # selector columns: col = c*12 + n  (c<12, n<12)

KE = E // P          # 6
KH = HID // P        # 12
OT = S // P          # 8 token tiles

_CACHE = {}
DEBUG_DUMPS = False


def _build_selector_T():
    """A^T with shape (S, ACOLS); column c*12 + (n//2) + 6*(n%2) selects
    tokens s whose chunk c belongs to head n, i.e. (12 s + c) // 1024 == n.
    Within each c-group, even heads occupy cols 0..5 and odd heads 6..11 so
    stage 2 can write even/odd heads to partition halves with contiguous
    rhs slices."""
    at = np.zeros((S, ACOLS), np.float32)
    s = np.arange(S)
    for c in range(HEADS):
        n = (HEADS * s + c) // S
        at[s, c * 12 + (n // 2) + 6 * (n % 2)] = 1.0
    return at


def _split_multi_waits(m):
    """Hoist all-but-one sync waits of each instruction onto preceding
    single-wait EventSemaphore instructions on the same engine.  Several TPB
    instruction structs (LDWEIGHTS for 4-byte matmuls, ctrl no-operand) carry
    only one sync-wait slot, and walrus codegen errors on more."""
    counter = [0]

    def fix_block(blk):
        out = []
        for inst in blk.get("instructions", []):
            si = inst.get("sync_info")
            waits = (si or {}).get("on_wait") or []
            if si and len(waits) > 1 and inst.get("opcode") != "EventSemaphore":
                for w in waits[:-1]:
                    counter[0] += 1
                    out.append({
                        "debug": inst.get("debug", 0), "engine": inst["engine"],
                        "ins": [], "outs": [], "name": f"I-wsplit-{counter[0]}",
                        "opcode": "EventSemaphore",
                        "sync_info": {"on_update": [], "on_wait": [w]},
                    })
                si["on_wait"] = waits[-1:]
            out.append(inst)
        blk["instructions"] = out
        for sub in blk.get("blocks", []):
            fix_block(sub)

    for fn in m["functions"]:
        for blk in fn["blocks"]:
            fix_block(blk)
    return m


def _build_bass():
    import json
    import concourse.bass as bass
    import concourse.mybir as mybir
    import concourse.tile as tile

    f32 = mybir.dt.float32
    bf16 = mybir.dt.bfloat16
    AX = mybir.AxisListType.X
    OP = mybir.AluOpType
    AF = mybir.ActivationFunctionType

    nc = bass.Bass(trn_type="TRN2")

    x_d = nc.declare_dram_parameter("x", [S, E], bf16, isOutput=False)
    wv_d = nc.declare_dram_parameter("Wv", [E, E], bf16, isOutput=False)
    w1_d = nc.declare_dram_parameter("W1", [E, HID], bf16, isOutput=False)
    w2_d = nc.declare_dram_parameter("W2", [HID, E], bf16, isOutput=False)
    at_d = nc.declare_dram_parameter("AT", [S, ACOLS], bf16, isOutput=False)
    idn_d = nc.declare_dram_parameter("IDN", [P, P], bf16, isOutput=False)
    # AUX fp32 [128, 24]: cols 0:6 g1col, 6:12 be1col, 12:24 b1col
    aux_d = nc.declare_dram_parameter("AUX", [P, 24], f32, isOutput=False)
    # BVEC bf16 [3*768]: b2 | g2 | beta2 (partition-broadcast on load)
    bv_d = nc.declare_dram_parameter("BVEC", [3 * E], bf16, isOutput=False)
    out_d = nc.declare_dram_parameter("out", [S, E], bf16, isOutput=True)
    if DEBUG_DUMPS:
        dbg = {
            "XT": nc.declare_dram_parameter("XT", [KE * P, S], bf16, isOutput=True),
            "YT": nc.declare_dram_parameter("YT", [KE * P, ACOLS], bf16, isOutput=True),
            "LNCOL": nc.declare_dram_parameter("LNCOL", [P, KE], bf16, isOutput=True),
            "HB1": nc.declare_dram_parameter("HB1", [P, KH], f32, isOutput=True),
            "G": nc.declare_dram_parameter("G", [KH * P, S], bf16, isOutput=True),
            "M": nc.declare_dram_parameter("M", [P, OT, E], bf16, isOutput=True),
            "MRB": nc.declare_dram_parameter("MRB", [P, 2], f32, isOutput=True),
            "LRAW": nc.declare_dram_parameter("LRAW", [P, KE], f32, isOutput=True),
        }

    x_v = x_d[:].rearrange("(o p) f -> p o f", p=P)      # (128, 8, 768)
    out_v = out_d[:].rearrange("(o p) f -> p o f", p=P)
    at_v = at_d[:].rearrange("(o p) f -> p o f", p=P)    # (128, 8, 192)
    wv_v = wv_d[:].rearrange("(k p) f -> p k f", p=P)    # (128, 6, 768)
    w1_v = w1_d[:].rearrange("(k p) f -> p k f", p=P)    # (128, 6, 1536)
    w2_v = w2_d[:].rearrange("(k p) f -> p k f", p=P)    # (128, 12, 768)

    with tile.TileContext(nc) as tc:
        with (
            tc.tile_pool(name="w1p", bufs=1) as w1p,
            tc.tile_pool(name="w2p", bufs=1) as w2p,
            tc.tile_pool(name="xp", bufs=1) as xp,
            tc.tile_pool(name="xtp", bufs=1) as xtp,
            tc.tile_pool(name="wvp", bufs=1) as wvp,
            tc.tile_pool(name="atp", bufs=1) as atp,
            tc.tile_pool(name="ytp", bufs=1) as ytp,
            tc.tile_pool(name="gp", bufs=1) as gp,
            tc.tile_pool(name="mp", bufs=1) as mp,
            tc.tile_pool(name="xbp", bufs=1) as xbp,
            tc.tile_pool(name="consts", bufs=1) as consts,
            tc.tile_pool(name="small", bufs=1) as small,
            tc.tile_pool(name="stat", bufs=4) as statp,
        ):
            # ---- DMA issue ------------------------------------------------
            # Pool/SWDGE channel (runs parallel to the HW DGE channel):
            idn = consts.tile([P, P], bf16)
            nc.gpsimd.dma_start(out=idn, in_=idn_d[:])
            at_sb = atp.tile([P, OT, ACOLS], bf16)
            nc.gpsimd.dma_start(out=at_sb, in_=at_v)
            wv_sb = wvp.tile([P, KE, E], bf16)
            nc.gpsimd.dma_start(out=wv_sb, in_=wv_v)
            aux = consts.tile([P, 24], f32)
            nc.gpsimd.dma_start(out=aux, in_=aux_d[:])
            g1col = aux[:, 0:KE]
            be1col = aux[:, KE:2 * KE]
            b1col = aux[:, 2 * KE:2 * KE + KH]
            bvecs = consts.tile([P, 3, E], bf16)
            nc.gpsimd.dma_start(out=bvecs, in_=bv_d[:].partition_broadcast(P))
            b2b = bvecs[:, 0, :]
            g2b = bvecs[:, 1, :]
            be2b = bvecs[:, 2, :]

            # HW DGE channel (serial DMA_ENGINES resource): x first (needed
            # by the PE transposes), then W1 (stage 5), then W2 (stage 6).
            x_sb = xp.tile([P, OT, E], bf16)
            nc.sync.dma_start(out=x_sb[:, 0:4, :], in_=x_v[:, 0:4, :])
            nc.sync.dma_start(out=x_sb[:, 4:8, :], in_=x_v[:, 4:8, :])
            w1_sb = w1p.tile([P, KE, HID], bf16)
            nc.sync.dma_start(out=w1_sb, in_=w1_v)
            w2_sb = w2p.tile([P, KH, E], bf16)
            nc.sync.dma_start(out=w2_sb, in_=w2_v)

            eps_sb = consts.tile([P, 1], f32)
            nc.vector.memset(eps_sb, EPS)
            ones128 = consts.tile([P, 1], f32)
            nc.vector.memset(ones128, 1.0)

            xt_sb = xtp.tile([P, KE, S], bf16)
            yt_sb = ytp.tile([P, KE, ACOLS], bf16)

            with (
                tc.tile_pool(name="pst", bufs=3, space="PSUM") as pstp,
                tc.tile_pool(name="ps1", bufs=3, space="PSUM") as ps1p,
                tc.tile_pool(name="pssA", bufs=1, space="PSUM") as pssA,
            ):
                # ---- x transposes on PE (xT feeds stage 5 directly) ------
                for o in range(OT):
                    for j in range(KE):
                        pst = pstp.tile([P, P], bf16, tag="pst")
                        nc.tensor.transpose(pst, x_sb[:, o, j * P:(j + 1) * P], idn)
                        nc.scalar.activation(
                            out=xt_sb[:, j, o * P:(o + 1) * P], in_=pst, func=AF.Copy,
                        )

                # ---- stage 1: Y^T = x^T @ A^T  (768 x 192), 2 waves ------
                for wave in (0, 3):
                    ps1t = [ps1p.tile([P, 512], f32, tag="ps1",
                                      name=f"ps1_{wave}_{i3}")
                            for i3 in range(3)]
                    for o in range(OT):
                        for i3 in range(3):
                            i = wave + i3
                            nc.tensor.matmul(
                                ps1t[i3][:, :ACOLS],
                                x_sb[:, o, i * P:(i + 1) * P],
                                at_sb[:, o, :],
                                start=(o == 0),
                                stop=(o == OT - 1),
                            )
                    for i3 in range(3):
                        nc.scalar.activation(out=yt_sb[:, wave + i3, :],
                                             in_=ps1t[i3][:, :ACOLS], func=AF.Copy)

                if DEBUG_DUMPS:
                    nc.gpsimd.dma_start(
                        out=dbg["XT"][:].rearrange("(k p) s -> p k s", p=P),
                        in_=xt_sb)
                    nc.gpsimd.dma_start(
                        out=dbg["YT"][:].rearrange("(k p) c -> p k c", p=P),
                        in_=yt_sb)

                # ---- stage 2: wT[two*64+d, j] = w[2j+two, d] directly -----
                # lhsT = Wv 64-col block (out partitions = d), rhs = the 6
                # even-head (or odd-head) selector columns of Y^T.
                ps_wT = pssA.tile([P, KE], f32)
                n_mm = 0
                for k in range(KE):
                    for c in range(HEADS):
                        n_mm += 1
                        first, last = n_mm == 1, n_mm == KE * HEADS
                        nc.tensor.matmul(
                            ps_wT[0:HD, :],
                            wv_sb[:, k, c * HD:(c + 1) * HD],
                            yt_sb[:, k, c * 12:c * 12 + 6],
                            start=first, stop=last, skip_group_check=True,
                        )
                        nc.tensor.matmul(
                            ps_wT[HD:P, :],
                            wv_sb[:, k, c * HD:(c + 1) * HD],
                            yt_sb[:, k, c * 12 + 6:(c + 1) * 12],
                            start=first, stop=last, skip_group_check=True,
                            tile_position=(0, 64),
                        )
                wcol = small.tile([P, KE], f32)
                nc.scalar.activation(out=wcol, in_=ps_wT, func=AF.Copy)

                # ---- stage 3: LN1 stats ------------------------------
                wsq = small.tile([P, KE], f32)
                nc.vector.tensor_mul(wsq, wcol, wcol)
                colsums = small.tile([P, 2], f32)
                nc.vector.tensor_reduce(out=colsums[:, 0:1], in_=wcol, axis=AX,
                                        op=OP.add)
                nc.vector.tensor_reduce(out=colsums[:, 1:2], in_=wsq, axis=AX,
                                        op=OP.add)
                ps_tot = pssA.tile([1, 2], f32)
                nc.tensor.matmul(ps_tot, ones128, colsums, start=True, stop=True)

                tots = small.tile([1, 2], f32)  # [mu, E[w^2]]
                nc.vector.tensor_scalar_mul(tots, ps_tot, 1.0 / E)
                mu2 = small.tile([1, 1], f32)
                nc.vector.tensor_mul(mu2, tots[:, 0:1], tots[:, 0:1])
                mr = small.tile([32, 2], f32)  # [mu, rstd] on partition 0
                nc.vector.tensor_sub(mr[:1, 1:2], tots[:, 1:2], mu2)  # var
                nc.scalar.activation(out=mr[:1, 1:2], in_=mr[:1, 1:2],
                                     func=AF.Sqrt, bias=eps_sb[:1])
                nc.vector.reciprocal(mr[:1, 1:2], mr[:1, 1:2])
                nc.vector.tensor_copy(mr[:1, 0:1], tots[:, 0:1])
                mrb = small.tile([P, 2], f32)
                for q in range(4):
                    nc.vector.stream_shuffle(mrb[32 * q:32 * (q + 1), :],
                                             mr[:, :], [0] * 32)

                # lncol[p, j] = LN1-affine of wcol (already in column layout)
                lncol = small.tile([P, KE], bf16)
                lnc_t = small.tile([P, KE], f32)
                if DEBUG_DUMPS:
                    nc.gpsimd.dma_start(out=dbg["LRAW"][:], in_=wcol)
                    nc.gpsimd.dma_start(out=dbg["MRB"][:], in_=mrb)
                nc.vector.tensor_scalar(lnc_t, wcol, mrb[:, 0:1],
                                        mrb[:, 1:2], OP.subtract, OP.mult)
                nc.vector.tensor_mul(lnc_t, lnc_t, g1col)
                nc.vector.tensor_add(lncol, lnc_t, be1col)
                if DEBUG_DUMPS:
                    nc.gpsimd.dma_start(out=dbg["LNCOL"][:], in_=lncol)

            # xb2[o] = x[o] + beta2 (Pool; final-residual operand)
            xb2 = xbp.tile([P, OT, E], bf16)
            for o in range(OT):
                nc.gpsimd.tensor_add(xb2[:, o, :], x_sb[:, o, :], be2b)

            g_sb = gp.tile([P, KH, S], bf16)
            m_sb = mp.tile([P, OT, E], bf16)

            with (
                tc.tile_pool(name="ps5", bufs=6, space="PSUM") as psp,
                tc.tile_pool(name="pssB", bufs=1, space="PSUM") as pssB,
            ):
                # ---- stage 5: H^T = W1^T @ x^T; G = gelu(H^T + hb1) ------
                # PE order: j2=0,1 matmuls first (cover the LN1 tail), then
                # the tiny hb matmuls, then the rest.
                s5ps = {}

                def s5_mm(j2):
                    psa = psp.tile([P, 512], f32, tag="ps")
                    psb = psp.tile([P, 512], f32, tag="ps")
                    for k in range(KE):
                        lhs = w1_sb[:, k, j2 * P:(j2 + 1) * P]
                        nc.tensor.matmul(psa, lhs, xt_sb[:, k, 0:512],
                                         start=(k == 0), stop=(k == KE - 1))
                        nc.tensor.matmul(psb, lhs, xt_sb[:, k, 512:1024],
                                         start=(k == 0), stop=(k == KE - 1))
                    s5ps[j2] = (psa, psb)

                def s5_act(j2):
                    psa, psb = s5ps.pop(j2)
                    nc.scalar.activation(out=g_sb[:, j2, 0:512], in_=psa,
                                         func=AF.Gelu, bias=hb1col[:, j2:j2 + 1])
                    nc.scalar.activation(out=g_sb[:, j2, 512:1024], in_=psb,
                                         func=AF.Gelu, bias=hb1col[:, j2:j2 + 1])

                s5_mm(0)
                s5_mm(1)

                # hb = W1^T @ lnvec  (+ b1) -> per-partition gelu bias
                ps_hb = pssB.tile([P, KH], f32)
                for j2 in range(KH):
                    for k in range(KE):
                        nc.tensor.matmul(
                            ps_hb[:, j2:j2 + 1],
                            w1_sb[:, k, j2 * P:(j2 + 1) * P],
                            lncol[:, k:k + 1],
                            start=(k == 0), stop=(k == KE - 1),
                        )
                hb1col = small.tile([P, KH], f32)
                nc.vector.tensor_add(hb1col, ps_hb, b1col)

                if DEBUG_DUMPS:
                    nc.gpsimd.dma_start(out=dbg["HB1"][:], in_=hb1col)

                s5_act(0)
                s5_act(1)
                for j2 in range(2, KH):
                    s5_mm(j2)
                    s5_act(j2)
                if DEBUG_DUMPS:
                    nc.gpsimd.dma_start(
                        out=dbg["G"][:].rearrange("(k p) s -> p k s", p=P),
                        in_=g_sb)

                # ---- stage 6: m = gelu(G^T @ W2 + b2); out = x + LN2(m) --
                for o in range(OT):
                    ps0 = psp.tile([P, 512], f32, tag="ps")
                    ps1 = psp.tile([P, 512], f32, tag="ps")
                    for k in range(KH):
                        lhs = g_sb[:, k, o * P:(o + 1) * P]
                        nc.tensor.matmul(ps0[:, :384], lhs, w2_sb[:, k, 0:384],
                                         start=(k == 0), stop=(k == KH - 1))
                        nc.tensor.matmul(ps1[:, :384], lhs, w2_sb[:, k, 384:768],
                                         start=(k == 0), stop=(k == KH - 1))
                    u = m_sb[:, o, :]
                    nc.vector.tensor_add(u[:, 0:384], ps0[:, :384], b2b[:, 0:384])
                    nc.vector.tensor_add(u[:, 384:768], ps1[:, :384], b2b[:, 384:768])
                    nc.scalar.activation(out=u, in_=u, func=AF.Gelu)
                    if DEBUG_DUMPS:
                        nc.gpsimd.dma_start(out=dbg["M"][:, o, :], in_=u)

                    stats = statp.tile([P, 3, 6], f32, tag="st")
                    for sub in range(3):
                        nc.vector.bn_stats(out=stats[:, sub, :],
                                           in_=u[:, sub * 256:(sub + 1) * 256])
                    mv = statp.tile([P, 2], f32, tag="mv")
                    nc.vector.bn_aggr(out=mv, in_=stats)
                    rstd = statp.tile([P, 1], f32, tag="rstd")
                    nc.scalar.activation(out=rstd, in_=mv[:, 1:2], func=AF.Sqrt,
                                         bias=eps_sb)
                    nc.vector.reciprocal(rstd, rstd)

                    nc.vector.tensor_scalar(u, u, mv[:, 0:1], rstd,
                                            OP.subtract, OP.mult)
                    nc.vector.tensor_mul(u, u, g2b)
                    nc.vector.tensor_add(u[:, 0:384], u[:, 0:384],
                                         xb2[:, o, 0:384])
                    nc.sync.dma_start(out=out_v[:, o, 0:384], in_=u[:, 0:384])
                    nc.vector.tensor_add(u[:, 384:768], u[:, 384:768],
                                         xb2[:, o, 384:768])
                    nc.scalar.dma_start(out=out_v[:, o, 384:768], in_=u[:, 384:768])

    m = json.loads(mybir.module_to_json_bytes(nc.m))
    m = _split_multi_waits(m)
    nc.m = mybir.module_from_json_bytes(json.dumps(m).encode())
    return nc


def _get_nc():
    if "nc" not in _CACHE:
        _CACHE["nc"] = _build_bass()
        _CACHE["at"] = _build_selector_T()
    return _CACHE["nc"]


def _run(inputs, trace=False):
    import ml_dtypes
    from concourse.bass_utils import run_bass_kernel_spmd

    nc = _get_nc()
    at = _CACHE["at"]
    bf16 = ml_dtypes.bfloat16

    def f32c(a):
        return np.ascontiguousarray(np.asarray(a), dtype=np.float32)

    def bfc(a):
        return np.ascontiguousarray(np.asarray(a, dtype=np.float32).astype(bf16))

    g1 = f32c(inputs["g1"])
    be1 = f32c(inputs["beta1"])
    b1 = f32c(inputs["b1"])
    aux = np.concatenate(
        [g1.reshape(KE, P).T, be1.reshape(KE, P).T, b1.reshape(KH, P).T], axis=1
    )  # (128, 24) fp32
    bvec = np.concatenate(
        [f32c(inputs["b2"]), f32c(inputs["g2"]), f32c(inputs["beta2"])]
    ).astype(bf16)

    shared = {
        "Wv": bfc(inputs["Wv"]),
        "W1": bfc(inputs["W1"]),
        "W2": bfc(inputs["W2"]),
        "AT": at.astype(bf16),
        "IDN": np.eye(P, dtype=np.float32).astype(bf16),
        "AUX": np.ascontiguousarray(aux, dtype=np.float32),
        "BVEC": bvec,
    }
    x = np.asarray(inputs["x"], dtype=np.float32).astype(bf16)
    in_maps = [dict(shared, x=np.ascontiguousarray(x[b])) for b in range(N_CORES)]
    res = run_bass_kernel_spmd(
        nc, in_maps, core_ids=list(range(N_CORES)), trace=trace,
        **({"trace_cores": list(range(N_CORES))} if trace else {}),
    )
    out = np.stack([r["out"].astype(np.float32) for r in res.results], axis=0)
    return out, res


def kernel(x, Wq=None, Wk=None, Wv=None, W1=None, b1=None, W2=None, b2=None,
           g1=None, beta1=None, g2=None, beta2=None):
    out, _ = _run(dict(x=x, Wv=Wv, W1=W1, b1=b1, W2=W2, b2=b2, g1=g1,
                       beta1=beta1, g2=g2, beta2=beta2))
    return out


def kernel_profiled(**inputs):
    out, res = _run(inputs, trace=True)
    return out, res
